# revision 41
# baseline (speedup 1.0000x reference)
"""GIN message-passing encoder (3 layers) on 8 Trainium2 NeuronCores.

Problem: x_{l+1} = relu(BN(relu((x + agg(x)) @ W1 + b1) @ W2 + b2)),
agg[b, d] = sum over edges (s -> d) of x[b, s]; output = stack of the 3
layer outputs, shape [3, 16, 1024, 256].

Strategy
--------
- Data parallel over batch: B=16 split as 2 batch elements per core.
- The scatter-add is a dense matmul against a host-built (N x N) matrix
  Bm[s, d] = I[s, d] + multiplicity(edge s -> d); the +x of GIN(eps=0)
  is the identity fold. Runs in fp8e4 DoubleRow (K=256/instruction).
- Eval-mode BatchNorm is folded into W2/b2 on the host; MLP matmuls in
  bf16.
- The matmul stream is the serial floor (~47us: 1 free-elem/cycle at
  2.4GHz; DoubleRow halves instruction count on the K=1024
  contraction), so the PE stream is scheduled gapless: per layer the
  order is S1(b0) S1(b1) S2(b0) S2(b1) S3(b0) S3(b1), and each
  stage's producers (DVE/ACT casts and relus) complete during
  unrelated PE work.
- Elementwise work is split between ACT and DVE so neither gates the
  PE: ACT does the ft0 m0-cast + half0 step2-relu + even-tp y-relu +
  odd-tp x-requant; DVE does the mirror set. (GPSIMD measured 3.6us
  per cast plus ~1us drains - unusable.)
- PSUM: step1 rotates 4x [P,512] bufs (4 banks; fine groups so each
  cast starts a quarter into S1 and banks recycle early); all
  step2/step3 groups share one 4-buf [P,512] pool (4 banks) so bank
  reuse never stalls.
- b2 bias enters step3's PSUM via a ones-matmul (lhsT=ones[128,128],
  rhs holds b2' pair on partition 0) in one N=512 instruction.
- Inputs are host-preswizzled; loads are spread across the scalar and
  sync queues in first-use order with per-kk completion semaphores so
  the layer-0 step-1 (kk-outermost there, halving the required bm
  bandwidth) unblocks progressively while the input stream lands.
  ones/b2z-zeros are built on-chip (DVE memset) instead of uploaded.
  Stores ride the sync queue as [P,4,F] half-batches; the final
  batch's ride the by-then-idle scalar queue in quarters.
- The node dimension is permuted (host-side, free) so PSUM partition p
  ends up holding 4 consecutive output nodes {4p..4p+3}: stores then
  write one 4KB contiguous HBM run per partition instead of 4x 1KB,
  roughly doubling effective store bandwidth. The permutation maps
  position nt*128+pp -> node 512*(nt//4) + 4*pp + nt%4 on the bm
  columns; bm rows / x carry the matching s-swizzle
  s(p,kk,r) = 512*(kk//2) + 4p + 2*(kk%2) + r so the DoubleRow chunks
  stay consistent and the on-device AP shapes are unchanged.
"""

import os

import numpy as np

BN_EPS = 1e-5

B, N, F = 16, 1024, 256
L = 3
NCORES = 8
BPC = B // NCORES  # batch elements per core
P = 128
NT = N // P   # 8 node tiles
FT = F // P   # 2 feature tiles
KK = N // 256  # 4 double-chunks of the contraction dim (DoubleRow K=256)
HALF = 512    # moving free-dim chunk
NH = N // HALF  # 2 halves of the node dim

_cache: dict = {}


def _build_nc():
    import concourse.bacc as bacc
    import concourse.mybir as mybir
    import concourse.tile as tile

    F32 = mybir.dt.float32
    F32R = mybir.dt.float32r
    F8 = mybir.dt.float8e4
    BF16 = mybir.dt.bfloat16
    Relu = mybir.ActivationFunctionType.Relu
    Copy = mybir.ActivationFunctionType.Copy
    Alu = mybir.AluOpType
    DR = mybir.MatmulPerfMode.DoubleRow

    nc = bacc.Bacc()

    x0hi_d = nc.dram_tensor("x0hi", [BPC, P, KK, 2, F], F8, kind="ExternalInput")
    bm_d = nc.dram_tensor("bm", [P, KK, 2, N], F8, kind="ExternalInput")
    w1_d = nc.dram_tensor("w1", [P, L, FT, F], BF16, kind="ExternalInput")
    w18_d = nc.dram_tensor("w18", [P, FT, F], F8, kind="ExternalInput")
    w2_d = nc.dram_tensor("w2", [P, L, FT, F], BF16, kind="ExternalInput")
    b1_d = nc.dram_tensor("b1", [P, L * FT], F32, kind="ExternalInput")
    b2_d = nc.dram_tensor("b2", [1, L, 2 * F], BF16, kind="ExternalInput")
    out_d = nc.dram_tensor("out", [L, BPC, N, F], F32R, kind="ExternalOutput")

    with tile.TileContext(nc) as tc:
        with (
            tc.tile_pool(name="const", bufs=1) as cpool,
            tc.tile_pool(name="x8", bufs=2) as xpool,
            tc.tile_pool(name="m0", bufs=2) as mpool,
            tc.tile_pool(name="h1", bufs=2) as hpool,
            tc.tile_pool(name="yt", bufs=4) as ypool,
            tc.tile_pool(name="pm0", bufs=4, space="PSUM") as pm0,
            tc.tile_pool(name="pmlp", bufs=4, space="PSUM") as pmlp,
        ):
            bm_sb = cpool.tile([P, KK, 2, N], F8)
            w1_sb = cpool.tile([P, L, FT, F], BF16)
            w18_sb = cpool.tile([P, FT, F], F8)
            w2_sb = cpool.tile([P, L, FT, F], BF16)
            b1_sb = cpool.tile([P, L * FT], F32)
            b2z_sb = cpool.tile([P, L, 2 * F], BF16)
            ones_sb = cpool.tile([P, P], BF16)

            xhi = xpool.tile([P, BPC, KK, 2, F], F8, tag="xhi")

            # Input DMAs. All queues stripe over the same 16 DMA
            # engines, so order matters globally, not per queue: the
            # first matmul's pieces (bm kk0 + x(b0) kk0) go first and
            # alone, then the bulk in first-use order.
            # per-kk pieces: the cold-start S1 consumes chunk kk as soon
            # as its own completion semaphore fires, instead of waiting
            # for the whole stream
            nc.sync.dma_start(bm_sb[:, 0:1], bm_d[:, 0:1])
            nc.scalar.dma_start(xhi[:, 0, 0:1], x0hi_d[0, :, 0:1])
            for kk in range(1, KK):
                nc.sync.dma_start(bm_sb[:, kk:kk + 1], bm_d[:, kk:kk + 1])
                nc.scalar.dma_start(xhi[:, 0, kk:kk + 1], x0hi_d[0, :, kk:kk + 1])
            nc.scalar.dma_start(xhi[:, 1, 0:1], x0hi_d[1, :, 0:1])
            nc.scalar.dma_start(xhi[:, 1, 1:4], x0hi_d[1, :, 1:4])
            nc.scalar.dma_start(w18_sb[:], w18_d[:])
            nc.scalar.dma_start(b1_sb[:], b1_d[:])
            nc.sync.dma_start(w2_sb[:], w2_d[:])
            nc.scalar.dma_start(w1_sb[:], w1_d[:])
            # ones and the zero rows of b2z are built on-chip (the DVE
            # is idle during the input stream); only b2' itself (3KB)
            # is uploaded, into partition 0.
            nc.vector.memset(ones_sb[:], 1.0)
            nc.vector.memset(b2z_sb[:], 0.0)
            nc.sync.dma_start(b2z_sb[0:1, :, :], b2_d[:])

            for l in range(L):
                last = l == L - 1
                if not last:
                    nxhi = xpool.tile([P, BPC, KK, 2, F], F8, tag="xhi")

                # ---- step 1: m0T = (A + I) @ x_q, fp8 DoubleRow ----
                # (l0, b0) runs kk-outermost: it consumes the still-
                # streaming bm at half the bandwidth (4 MMs per kk
                # chunk instead of 2). Steady state runs ft-outermost
                # so the psum group for ft0 completes early and its
                # cast overlaps the ft1 group's matmuls.
                m0t = [None, None]
                for b in range(BPC):
                    m0t[b] = mpool.tile([P, FT, N], F8 if l == 0 else BF16,
                                        tag="m0t", name=f"m0t{b}")
                    pss = {}
                    for ft in range(FT):
                        for half in range(NH):
                            pss[ft, half] = pm0.tile(
                                [P, HALF], F32, tag="pm0", name=f"ps{ft}{half}")
                    cold = l == 0
                    # fine [P,512] groups: each completes after 4 MMs, so
                    # its cast starts a quarter into S1 and the bank
                    # recycles early. cold (l0) runs kk-outermost to
                    # halve the bm stream bandwidth demand.
                    order = (
                        [(kk, ft, half) for kk in range(KK)
                         for ft in range(FT) for half in range(NH)]
                        if cold else
                        [(kk, ft, half) for ft in range(FT)
                         for half in range(NH) for kk in range(KK)]
                    )
                    for kk, ft, half in order:
                        nc.tensor.matmul(
                            pss[ft, half][:],
                            xhi[:, b, kk, :, ft * P:(ft + 1) * P],
                            bm_sb[:, kk, :,
                                  half * HALF:(half + 1) * HALF],
                            start=(kk == 0),
                            stop=(kk == KK - 1),
                            perf_mode=DR,
                        )
                    # PSUM->SBUF casts split across ACT (ft0) and DVE
                    # (ft1): either alone is slower than the PE
                    # producing the next group.
                    for half in range(NH):
                        nc.scalar.activation(
                            m0t[b][:, 0, half * HALF:(half + 1) * HALF],
                            pss[0, half][:], Copy)
                        nc.vector.tensor_copy(
                            m0t[b][:, 1, half * HALF:(half + 1) * HALF],
                            pss[1, half][:])

                h1t = [None, None]
                ysb = [[None, None], [None, None]]
                for b in range(BPC):
                    # ---- step 2: h1T = relu(W1^T-contract @ m0T + b1) ----
                    h1t[b] = hpool.tile([P, FT, N], BF16, tag="h1t", name=f"h1t{b}")
                    for gt in range(FT):
                        pg = [pmlp.tile([P, HALF], F32, tag="mlp", name=f"pg{h}")
                              for h in range(NH)]
                        if l == 0:
                            # fp8 DoubleRow: K=256 in one instruction;
                            # W1(l0) is x8 on the host, h1(l0) lives at
                            # 8x in bf16, b1(l0) x8 and W2(l0) /8
                            # compensate.
                            for half in range(NH):
                                nc.tensor.matmul(
                                    pg[half][:],
                                    w18_sb[:, :, gt * P:(gt + 1) * P],
                                    m0t[b][:, :,
                                           half * HALF:(half + 1) * HALF],
                                    start=True, stop=True,
                                    perf_mode=DR,
                                )
                        else:
                            for fk in range(FT):
                                for half in range(NH):
                                    nc.tensor.matmul(
                                        pg[half][:],
                                        w1_sb[:, l, fk, gt * P:(gt + 1) * P],
                                        m0t[b][:, fk,
                                               half * HALF:(half + 1) * HALF],
                                        start=(fk == 0),
                                        stop=(fk == FT - 1),
                                    )
                        # relu+bias split between ACT and DVE; alternate
                        # the halves per gt so step3's two producers for
                        # any node-half come from different engines
                        eng_a = (gt == 0)
                        acts = [(0, True), (1, False)] if eng_a else [(0, False), (1, True)]
                        for half, use_act in acts:
                            hs = h1t[b][:, gt, half * HALF:(half + 1) * HALF]
                            bb = b1_sb[:, l * FT + gt:l * FT + gt + 1]
                            if use_act:
                                nc.scalar.activation(hs, pg[half][:], Relu, bias=bb)
                            else:
                                nc.vector.tensor_scalar(
                                    hs, pg[half][:], bb, 0.0,
                                    op0=Alu.add, op1=Alu.max,
                                )

                for b in range(BPC):
                    # ---- step 3: y = relu(h1 @ W2' + b2') -> out + next x ----
                    for j in range(2):        # output half: nodes [512j, 512j+512)
                        ysb[b][j] = ypool.tile([P, 4, F], F32R, tag="y", name=f"y{b}{j}")
                        for t2 in range(2):   # tp = 2j + t2
                            tp = 2 * j + t2
                            ps3 = pmlp.tile([P, 2, F], F32, tag="mlp")
                            # seed b2' into PSUM via a ones-matmul
                            # (N=512 covers both node tiles), then
                            # accumulate both tiles' GEMMs.
                            nc.tensor.matmul(
                                ps3[:], ones_sb[:], b2z_sb[:, l, :],
                                start=True, stop=False, skip_group_check=True,
                            )
                            for jj in range(2):
                                nt = 2 * tp + jj
                                for gk in range(FT):
                                    nc.tensor.matmul(
                                        ps3[:, jj, :],
                                        h1t[b][:, gk, nt * P:(nt + 1) * P],
                                        w2_sb[:, l, gk, :],
                                        start=False,
                                        stop=(gk == FT - 1),
                                        skip_group_check=True,
                                    )
                            ydst = ysb[b][j][:, 2 * t2:2 * t2 + 2, :]
                            if t2 == 0:
                                nc.scalar.activation(ydst, ps3[:], Relu)
                                if not last:
                                    nc.vector.tensor_scalar(
                                        nxhi[:, b, tp, :, :], ps3[:],
                                        0.0, None, op0=Alu.max,
                                    )
                            else:
                                nc.vector.tensor_scalar(
                                    ydst, ps3[:], 0.0, None, op0=Alu.max,
                                )
                                if not last:
                                    nc.scalar.activation(
                                        nxhi[:, b, tp, :, :], ps3[:], Relu,
                                    )
                        # permuted node order: partition p holds nodes
                        # {512j + 4p + t}, one 4KB HBM run per partition.
                        # The very last batch's stores ride the (by then
                        # idle) scalar queue so the tail drain runs on
                        # two queues in parallel.
                        q = nc.scalar if (last and b == BPC - 1) else nc.sync
                        if last:
                            # drain the whole last layer in quarters
                            for t2 in range(2):
                                q.dma_start(
                                    out_d[l, b, j * 4 * P:(j + 1) * 4 * P, :]
                                    .rearrange("(p a t) f -> a p t f", a=2, t=2)[t2],
                                    ysb[b][j][:, 2 * t2:2 * t2 + 2, :],
                                )
                        else:
                            q.dma_start(
                                out_d[l, b, j * 4 * P:(j + 1) * 4 * P, :]
                                .rearrange("(p t) f -> p t f", p=P),
                                ysb[b][j][:],
                            )
                if not last:
                    xhi = nxhi

    nc.finalize()
    return nc


def kernel(h, edge_index, W1, b1, W2, b2, gamma, beta, run_mean, run_var):
    import ml_dtypes
    from concourse.bass_utils import run_bass_kernel_spmd

    f8 = ml_dtypes.float8_e4m3

    h = np.asarray(h, dtype=np.float32)
    edge_index = np.asarray(edge_index)
    W1 = np.asarray(W1, dtype=np.float32)
    b1 = np.asarray(b1, dtype=np.float32)
    W2 = np.asarray(W2, dtype=np.float32)
    b2 = np.asarray(b2, dtype=np.float32)
    gamma = np.asarray(gamma, dtype=np.float32)
    beta = np.asarray(beta, dtype=np.float32)
    run_mean = np.asarray(run_mean, dtype=np.float32)
    run_var = np.asarray(run_var, dtype=np.float32)

    # host-side preprocessing
    src = edge_index[0].astype(np.int64)
    dst = edge_index[1].astype(np.int64)
    bm = np.zeros((N, N), dtype=np.float32)
    np.add.at(bm, (src, dst), 1.0)
    bm[np.arange(N), np.arange(N)] += 1.0

    # node permutation: column position nt*128+pp holds node
    # 512*(nt//4) + 4*pp + nt%4, so the step-3 output partition p owns
    # nodes {4p..4p+3} of its 512-block (4KB contiguous store runs).
    # Rows (the contraction side) carry the matching s-swizzle
    # s(p, kk, r) = 512*(kk//2) + 4p + 2*(kk%2) + r.
    nt_i, pp_i = np.divmod(np.arange(N), P)
    colperm = 512 * (nt_i // 4) + 4 * pp_i + nt_i % 4
    p_i = np.arange(P)[:, None, None]
    kk_i = np.arange(KK)[None, :, None]
    r_i = np.arange(2)[None, None, :]
    srow = 512 * (kk_i // 2) + 4 * p_i + 2 * (kk_i % 2) + r_i  # [P, KK, 2]

    # fp8 exact for small integer counts; DoubleRow layout [P, KK, 2, N]
    bm8 = np.ascontiguousarray(bm.astype(f8)[srow][:, :, :, colperm])

    # x0 quantized to fp8 on the host, swizzled to [B, P, KK, 2, F]
    xhi8s = np.ascontiguousarray(h.astype(f8)[:, srow, :])

    inv = (gamma / np.sqrt(run_var + BN_EPS)).astype(np.float32)      # [L, F]
    w2f = (W2 * inv[:, None, :]).astype(np.float32)                   # [L, F, F]
    b2f = (b2 * inv + beta - run_mean * inv).astype(np.float32)       # [L, F]

    # weights swizzled to [P, L, FT, F] (contraction chunk on partitions),
    # bf16 to halve the upload (adds ~2e-3 to the error budget)
    bf16 = ml_dtypes.bfloat16
    w18 = np.ascontiguousarray(
        (W1[0] * 8.0).reshape(FT, P, F).transpose(1, 0, 2).astype(f8)
    )
    w1s = np.ascontiguousarray(
        W1.reshape(L, FT, P, F).transpose(2, 0, 1, 3).astype(bf16)
    )
    w2c = w2f.copy()
    w2c[0] /= 8.0
    w2s = np.ascontiguousarray(
        w2c.reshape(L, FT, P, F).transpose(2, 0, 1, 3).astype(bf16)
    )
    # b1 as per-partition scalars: [P, L*FT]; l0 slot carries the x8
    b1c = b1.copy()
    b1c[0] *= 8.0
    b1r = np.ascontiguousarray(
        b1c.reshape(L, FT, P).transpose(2, 0, 1).reshape(P, L * FT)
    )
    # b2' (duplicated pair), uploaded to partition 0 only; the
    # ones-matmul broadcasts it into step3's PSUM
    b2r = np.concatenate([b2f, b2f], axis=1).astype(bf16)[None]

    if "nc" not in _cache:
        _cache["nc"] = _build_nc()
    nc = _cache["nc"]

    in_maps = []
    for c in range(NCORES):
        in_maps.append({
            "x0hi": np.ascontiguousarray(xhi8s[c * BPC:(c + 1) * BPC]),
            "bm": bm8,
            "w1": w1s,
            "w18": w18,
            "w2": w2s,
            "b1": b1r,
            "b2": b2r,
        })

    trace = os.environ.get("KERNEL_TRACE") == "1"
    res = run_bass_kernel_spmd(
        nc, in_maps, core_ids=list(range(NCORES)), trace=trace
    )
    _cache["last_results"] = res
    return np.concatenate([r["out"] for r in res.results], axis=1)


# revision 42
# speedup vs baseline: 1.0212x; 1.0212x over previous
"""GIN message-passing encoder (3 layers) on 8 Trainium2 NeuronCores.

Problem: x_{l+1} = relu(BN(relu((x + agg(x)) @ W1 + b1) @ W2 + b2)),
agg[b, d] = sum over edges (s -> d) of x[b, s]; output = stack of the 3
layer outputs, shape [3, 16, 1024, 256].

Strategy
--------
- Data parallel over batch: B=16 split as 2 batch elements per core.
- The scatter-add is a dense matmul against a host-built (N x N) matrix
  Bm[s, d] = I[s, d] + multiplicity(edge s -> d); the +x of GIN(eps=0)
  is the identity fold. Runs in fp8e4 DoubleRow (K=256/instruction).
- Eval-mode BatchNorm is folded into W2/b2 on the host; MLP matmuls in
  bf16.
- The matmul stream is the serial floor (~47us: 1 free-elem/cycle at
  2.4GHz; DoubleRow halves instruction count on the K=1024
  contraction), so the PE stream is scheduled gapless: per layer the
  order is S1(b0) S1(b1) S2(b0) S2(b1) S3(b0) S3(b1), and each
  stage's producers (DVE/ACT casts and relus) complete during
  unrelated PE work.
- Elementwise work is split between ACT and DVE so neither gates the
  PE: ACT does the ft0 m0-cast + half0 step2-relu + even-tp y-relu +
  odd-tp x-requant; DVE does the mirror set. (GPSIMD measured 3.6us
  per cast plus ~1us drains - unusable.)
- PSUM: step1 rotates 4x [P,512] bufs (4 banks; fine groups so each
  cast starts a quarter into S1 and banks recycle early); all
  step2/step3 groups share one 4-buf [P,512] pool (4 banks) so bank
  reuse never stalls.
- b2 bias enters step3's PSUM via a ones-matmul (lhsT=ones[128,128],
  rhs holds b2' pair on partition 0) in one N=512 instruction.
- Inputs are host-preswizzled; loads are spread across the scalar and
  sync queues in first-use order with per-kk completion semaphores so
  the layer-0 step-1 (kk-outermost there, halving the required bm
  bandwidth) unblocks progressively while the input stream lands.
  ones/b2z-zeros are built on-chip (DVE memset) instead of uploaded.
  Stores ride the sync queue as [P,4,F] half-batches; the final
  batch's ride the by-then-idle scalar queue in quarters.
- The node dimension is permuted (host-side, free) so PSUM partition p
  ends up holding 4 consecutive output nodes {4p..4p+3}: stores then
  write one 4KB contiguous HBM run per partition instead of 4x 1KB,
  roughly doubling effective store bandwidth. The permutation maps
  position nt*128+pp -> node 512*(nt//4) + 4*pp + nt%4 on the bm
  columns; bm rows / x carry the matching s-swizzle
  s(p,kk,r) = 512*(kk//2) + 4p + 2*(kk%2) + r so the DoubleRow chunks
  stay consistent and the on-device AP shapes are unchanged.
"""

import os

import numpy as np

BN_EPS = 1e-5

B, N, F = 16, 1024, 256
L = 3
NCORES = 8
BPC = B // NCORES  # batch elements per core
P = 128
NT = N // P   # 8 node tiles
FT = F // P   # 2 feature tiles
KK = N // 256  # 4 double-chunks of the contraction dim (DoubleRow K=256)
HALF = 512    # moving free-dim chunk
NH = N // HALF  # 2 halves of the node dim

_cache: dict = {}


def _build_nc():
    import concourse.bacc as bacc
    import concourse.mybir as mybir
    import concourse.tile as tile

    F32 = mybir.dt.float32
    F32R = mybir.dt.float32r
    F8 = mybir.dt.float8e4
    BF16 = mybir.dt.bfloat16
    Relu = mybir.ActivationFunctionType.Relu
    Copy = mybir.ActivationFunctionType.Copy
    Alu = mybir.AluOpType
    DR = mybir.MatmulPerfMode.DoubleRow

    nc = bacc.Bacc()

    x0hi_d = nc.dram_tensor("x0hi", [BPC, P, KK, 2, F], F8, kind="ExternalInput")
    bm_d = nc.dram_tensor("bm", [P, KK, 2, N], F8, kind="ExternalInput")
    w1_d = nc.dram_tensor("w1", [P, L, FT, F], BF16, kind="ExternalInput")
    w18_d = nc.dram_tensor("w18", [P, FT, F], F8, kind="ExternalInput")
    w2_d = nc.dram_tensor("w2", [P, L, FT, F], BF16, kind="ExternalInput")
    b1_d = nc.dram_tensor("b1", [P, L * FT], F32, kind="ExternalInput")
    b2_d = nc.dram_tensor("b2", [1, L, 2 * F], BF16, kind="ExternalInput")
    out_d = nc.dram_tensor("out", [L, BPC, N, F], F32R, kind="ExternalOutput")

    with tile.TileContext(nc) as tc:
        with (
            tc.tile_pool(name="const", bufs=1) as cpool,
            tc.tile_pool(name="x8", bufs=2) as xpool,
            tc.tile_pool(name="m0", bufs=2) as mpool,
            tc.tile_pool(name="h1", bufs=2) as hpool,
            tc.tile_pool(name="yt", bufs=4) as ypool,
            tc.tile_pool(name="pm0", bufs=4, space="PSUM") as pm0,
            tc.tile_pool(name="pmlp", bufs=4, space="PSUM") as pmlp,
        ):
            bm_sb = cpool.tile([P, KK, 2, N], F8)
            w1_sb = cpool.tile([P, L, FT, F], BF16)
            w18_sb = cpool.tile([P, FT, F], F8)
            w2_sb = cpool.tile([P, L, FT, F], BF16)
            b1_sb = cpool.tile([P, L * FT], F32)
            b2z_sb = cpool.tile([P, L, 2 * F], BF16)
            ones_sb = cpool.tile([P, P], BF16)

            xhi = xpool.tile([P, BPC, KK, 2, F], F8, tag="xhi")

            # Input DMAs. All queues stripe over the same 16 DMA
            # engines, so order matters globally, not per queue: the
            # first matmul's pieces (bm kk0 + x(b0) kk0) go first and
            # alone, then the bulk in first-use order.
            # per-kk pieces: the cold-start S1 consumes chunk kk as soon
            # as its own completion semaphore fires, instead of waiting
            # for the whole stream
            nc.sync.dma_start(bm_sb[:, 0:1], bm_d[:, 0:1])
            nc.scalar.dma_start(xhi[:, 0, 0:1], x0hi_d[0, :, 0:1])
            for kk in range(1, KK):
                nc.sync.dma_start(bm_sb[:, kk:kk + 1], bm_d[:, kk:kk + 1])
                nc.scalar.dma_start(xhi[:, 0, kk:kk + 1], x0hi_d[0, :, kk:kk + 1])
            nc.scalar.dma_start(xhi[:, 1, 0:1], x0hi_d[1, :, 0:1])
            nc.scalar.dma_start(xhi[:, 1, 1:4], x0hi_d[1, :, 1:4])
            nc.scalar.dma_start(w18_sb[:], w18_d[:])
            nc.scalar.dma_start(b1_sb[:], b1_d[:])
            nc.sync.dma_start(w2_sb[:], w2_d[:])
            nc.scalar.dma_start(w1_sb[:], w1_d[:])
            # ones and the zero rows of b2z are built on-chip (the DVE
            # is idle during the input stream); only b2' itself (3KB)
            # is uploaded, into partition 0.
            nc.vector.memset(ones_sb[:], 1.0)
            nc.vector.memset(b2z_sb[:], 0.0)
            nc.sync.dma_start(b2z_sb[0:1, :, :], b2_d[:])

            for l in range(L):
                last = l == L - 1
                if not last:
                    nxhi = xpool.tile([P, BPC, KK, 2, F], F8, tag="xhi")

                # ---- step 1: m0T = (A + I) @ x_q, fp8 DoubleRow ----
                # (l0, b0) runs kk-outermost: it consumes the still-
                # streaming bm at half the bandwidth (4 MMs per kk
                # chunk instead of 2). Steady state runs ft-outermost
                # so the psum group for ft0 completes early and its
                # cast overlaps the ft1 group's matmuls.
                m0t = [None, None]
                for b in range(BPC):
                    m0t[b] = mpool.tile([P, FT, N], F8 if l == 0 else BF16,
                                        tag="m0t", name=f"m0t{b}")
                    pss = {}
                    for ft in range(FT):
                        for half in range(NH):
                            pss[ft, half] = pm0.tile(
                                [P, HALF], F32, tag="pm0", name=f"ps{ft}{half}")
                    cold = l == 0
                    # fine [P,512] groups: each completes after 4 MMs, so
                    # its cast starts a quarter into S1 and the bank
                    # recycles early. cold (l0) runs kk-outermost to
                    # halve the bm stream bandwidth demand.
                    order = (
                        [(kk, ft, half) for kk in range(KK)
                         for ft in range(FT) for half in range(NH)]
                        if cold else
                        [(kk, ft, half) for ft in range(FT)
                         for half in range(NH) for kk in range(KK)]
                    )
                    for kk, ft, half in order:
                        nc.tensor.matmul(
                            pss[ft, half][:],
                            xhi[:, b, kk, :, ft * P:(ft + 1) * P],
                            bm_sb[:, kk, :,
                                  half * HALF:(half + 1) * HALF],
                            start=(kk == 0),
                            stop=(kk == KK - 1),
                            perf_mode=DR,
                        )
                    # PSUM->SBUF casts split across ACT (ft0) and DVE
                    # (ft1): either alone is slower than the PE
                    # producing the next group.
                    for half in range(NH):
                        nc.scalar.activation(
                            m0t[b][:, 0, half * HALF:(half + 1) * HALF],
                            pss[0, half][:], Copy)
                        nc.vector.tensor_copy(
                            m0t[b][:, 1, half * HALF:(half + 1) * HALF],
                            pss[1, half][:])

                h1t = [None, None]
                ysb = [[None, None], [None, None]]
                for b in range(BPC):
                    # ---- step 2: h1T = relu(W1^T-contract @ m0T + b1) ----
                    h1t[b] = hpool.tile([P, FT, N], BF16, tag="h1t", name=f"h1t{b}")
                    for gt in range(FT):
                        pg = [pmlp.tile([P, HALF], F32, tag="mlp", name=f"pg{h}")
                              for h in range(NH)]
                        if l == 0:
                            # fp8 DoubleRow: K=256 in one instruction;
                            # W1(l0) is x8 on the host, h1(l0) lives at
                            # 8x in bf16, b1(l0) x8 and W2(l0) /8
                            # compensate.
                            for half in range(NH):
                                nc.tensor.matmul(
                                    pg[half][:],
                                    w18_sb[:, :, gt * P:(gt + 1) * P],
                                    m0t[b][:, :,
                                           half * HALF:(half + 1) * HALF],
                                    start=True, stop=True,
                                    perf_mode=DR,
                                )
                        else:
                            for fk in range(FT):
                                for half in range(NH):
                                    nc.tensor.matmul(
                                        pg[half][:],
                                        w1_sb[:, l, fk, gt * P:(gt + 1) * P],
                                        m0t[b][:, fk,
                                               half * HALF:(half + 1) * HALF],
                                        start=(fk == 0),
                                        stop=(fk == FT - 1),
                                    )
                        # relu+bias split between ACT and DVE; alternate
                        # the halves per gt so step3's two producers for
                        # any node-half come from different engines
                        eng_a = (gt == 0)
                        acts = [(0, True), (1, False)] if eng_a else [(0, False), (1, True)]
                        for half, use_act in acts:
                            hs = h1t[b][:, gt, half * HALF:(half + 1) * HALF]
                            bb = b1_sb[:, l * FT + gt:l * FT + gt + 1]
                            if use_act:
                                nc.scalar.activation(hs, pg[half][:], Relu, bias=bb)
                            else:
                                nc.vector.tensor_scalar(
                                    hs, pg[half][:], bb, 0.0,
                                    op0=Alu.add, op1=Alu.max,
                                )

                for b in range(BPC):
                    # ---- step 3: y = relu(h1 @ W2' + b2') -> out + next x ----
                    for j in range(2):        # output half: nodes [512j, 512j+512)
                        ysb[b][j] = ypool.tile([P, 4, F], F32R, tag="y", name=f"y{b}{j}")
                        for t2 in range(2):   # tp = 2j + t2
                            tp = 2 * j + t2
                            ps3 = pmlp.tile([P, 2, F], F32, tag="mlp")
                            # seed b2' into PSUM via a ones-matmul
                            # (N=512 covers both node tiles), then
                            # accumulate both tiles' GEMMs.
                            nc.tensor.matmul(
                                ps3[:], ones_sb[:], b2z_sb[:, l, :],
                                start=True, stop=False, skip_group_check=True,
                            )
                            for jj in range(2):
                                nt = 2 * tp + jj
                                for gk in range(FT):
                                    nc.tensor.matmul(
                                        ps3[:, jj, :],
                                        h1t[b][:, gk, nt * P:(nt + 1) * P],
                                        w2_sb[:, l, gk, :],
                                        start=False,
                                        stop=(gk == FT - 1),
                                        skip_group_check=True,
                                    )
                            # the x-requant is the next layer's S1
                            # dependency, so it goes ahead of the y-relu
                            # on its engine (stores have drain slack)
                            ydst = ysb[b][j][:, 2 * t2:2 * t2 + 2, :]
                            if t2 == 0:
                                if not last:
                                    nc.vector.tensor_scalar(
                                        nxhi[:, b, tp, :, :], ps3[:],
                                        0.0, None, op0=Alu.max,
                                    )
                                nc.scalar.activation(ydst, ps3[:], Relu)
                            else:
                                if not last:
                                    nc.scalar.activation(
                                        nxhi[:, b, tp, :, :], ps3[:], Relu,
                                    )
                                nc.vector.tensor_scalar(
                                    ydst, ps3[:], 0.0, None, op0=Alu.max,
                                )
                        # permuted node order: partition p holds nodes
                        # {512j + 4p + t}, one 4KB HBM run per partition.
                        # The very last batch's stores ride the (by then
                        # idle) scalar queue so the tail drain runs on
                        # two queues in parallel.
                        q = nc.scalar if (last and b == BPC - 1) else nc.sync
                        if last:
                            # drain the whole last layer in quarters
                            for t2 in range(2):
                                q.dma_start(
                                    out_d[l, b, j * 4 * P:(j + 1) * 4 * P, :]
                                    .rearrange("(p a t) f -> a p t f", a=2, t=2)[t2],
                                    ysb[b][j][:, 2 * t2:2 * t2 + 2, :],
                                )
                        else:
                            q.dma_start(
                                out_d[l, b, j * 4 * P:(j + 1) * 4 * P, :]
                                .rearrange("(p t) f -> p t f", p=P),
                                ysb[b][j][:],
                            )
                if not last:
                    xhi = nxhi

    nc.finalize()
    return nc


def kernel(h, edge_index, W1, b1, W2, b2, gamma, beta, run_mean, run_var):
    import ml_dtypes
    from concourse.bass_utils import run_bass_kernel_spmd

    f8 = ml_dtypes.float8_e4m3

    h = np.asarray(h, dtype=np.float32)
    edge_index = np.asarray(edge_index)
    W1 = np.asarray(W1, dtype=np.float32)
    b1 = np.asarray(b1, dtype=np.float32)
    W2 = np.asarray(W2, dtype=np.float32)
    b2 = np.asarray(b2, dtype=np.float32)
    gamma = np.asarray(gamma, dtype=np.float32)
    beta = np.asarray(beta, dtype=np.float32)
    run_mean = np.asarray(run_mean, dtype=np.float32)
    run_var = np.asarray(run_var, dtype=np.float32)

    # host-side preprocessing
    src = edge_index[0].astype(np.int64)
    dst = edge_index[1].astype(np.int64)
    bm = np.zeros((N, N), dtype=np.float32)
    np.add.at(bm, (src, dst), 1.0)
    bm[np.arange(N), np.arange(N)] += 1.0

    # node permutation: column position nt*128+pp holds node
    # 512*(nt//4) + 4*pp + nt%4, so the step-3 output partition p owns
    # nodes {4p..4p+3} of its 512-block (4KB contiguous store runs).
    # Rows (the contraction side) carry the matching s-swizzle
    # s(p, kk, r) = 512*(kk//2) + 4p + 2*(kk%2) + r.
    nt_i, pp_i = np.divmod(np.arange(N), P)
    colperm = 512 * (nt_i // 4) + 4 * pp_i + nt_i % 4
    p_i = np.arange(P)[:, None, None]
    kk_i = np.arange(KK)[None, :, None]
    r_i = np.arange(2)[None, None, :]
    srow = 512 * (kk_i // 2) + 4 * p_i + 2 * (kk_i % 2) + r_i  # [P, KK, 2]

    # fp8 exact for small integer counts; DoubleRow layout [P, KK, 2, N]
    bm8 = np.ascontiguousarray(bm.astype(f8)[srow][:, :, :, colperm])

    # x0 quantized to fp8 on the host, swizzled to [B, P, KK, 2, F]
    xhi8s = np.ascontiguousarray(h.astype(f8)[:, srow, :])

    inv = (gamma / np.sqrt(run_var + BN_EPS)).astype(np.float32)      # [L, F]
    w2f = (W2 * inv[:, None, :]).astype(np.float32)                   # [L, F, F]
    b2f = (b2 * inv + beta - run_mean * inv).astype(np.float32)       # [L, F]

    # weights swizzled to [P, L, FT, F] (contraction chunk on partitions),
    # bf16 to halve the upload (adds ~2e-3 to the error budget)
    bf16 = ml_dtypes.bfloat16
    w18 = np.ascontiguousarray(
        (W1[0] * 8.0).reshape(FT, P, F).transpose(1, 0, 2).astype(f8)
    )
    w1s = np.ascontiguousarray(
        W1.reshape(L, FT, P, F).transpose(2, 0, 1, 3).astype(bf16)
    )
    w2c = w2f.copy()
    w2c[0] /= 8.0
    w2s = np.ascontiguousarray(
        w2c.reshape(L, FT, P, F).transpose(2, 0, 1, 3).astype(bf16)
    )
    # b1 as per-partition scalars: [P, L*FT]; l0 slot carries the x8
    b1c = b1.copy()
    b1c[0] *= 8.0
    b1r = np.ascontiguousarray(
        b1c.reshape(L, FT, P).transpose(2, 0, 1).reshape(P, L * FT)
    )
    # b2' (duplicated pair), uploaded to partition 0 only; the
    # ones-matmul broadcasts it into step3's PSUM
    b2r = np.concatenate([b2f, b2f], axis=1).astype(bf16)[None]

    if "nc" not in _cache:
        _cache["nc"] = _build_nc()
    nc = _cache["nc"]

    in_maps = []
    for c in range(NCORES):
        in_maps.append({
            "x0hi": np.ascontiguousarray(xhi8s[c * BPC:(c + 1) * BPC]),
            "bm": bm8,
            "w1": w1s,
            "w18": w18,
            "w2": w2s,
            "b1": b1r,
            "b2": b2r,
        })

    trace = os.environ.get("KERNEL_TRACE") == "1"
    res = run_bass_kernel_spmd(
        nc, in_maps, core_ids=list(range(NCORES)), trace=trace
    )
    _cache["last_results"] = res
    return np.concatenate([r["out"] for r in res.results], axis=1)
